# revision 19
# baseline (speedup 1.0000x reference)
"""GridToStation Trainium2 kernel (bf16 patch-gather version).

Strategy:
  - Host: transpose grid (C,H,W) -> (H,W,C), cast bf16, and build per-core
    2x2 PATCH tables: row (y, xl) = [g[y,x], g[y,x1], g[y1,x], g[y1,x1]]
    (1024 bf16 = 2KB), where each core's table is a 212-column lon window
    around its 180-column band (x1/y1 border-clipped at build time).
  - Host: replicate the reference's f32 index math; sort stations by ix0 and
    cut every N/8 -> EXACTLY 2048 stations per core (fits each core's window
    because station lon is near-uniform; asserted). No padding anywhere.
  - Device (per core, SPMD over 8 cores), 16 tiles of 128 stations:
      * one indirect (gather) DMA per tile pulls 128 patch rows (2KB each).
        One descriptor per station is the minimum serial GpSimd/SWDGE work
        (~1.1us per call fixed) -- this is the kernel's critical path.
      * bilinear combine in [station, C] bf16: ACT seed (scale=c00) + 3 DVE
        scalar_tensor_tensor fused multiply-adds.
      * PE transpose (bf16) -> PSUM f32 -> ACT copy to bf16 x^T tiles.
      * MLP in [C, station] layout, groups of 512 stations: W1/W2 bf16
        matmuls (PSUM f32), GELU+b1 on ACT, b2 add on DVE, bf16 out.
      * per-group output DMA (overlapped), 2x[128,512] bf16 per group.
  - Host: upcast bf16 -> f32 and inverse-permute to original station order.
"""

import os

import numpy as np
import ml_dtypes

B, C, H, W, N = 1, 256, 721, 1440, 16384
NCORES = 8
COLS = W // NCORES  # 180 owned columns per core
FX = 16  # window flex columns each side
WT = COLS + 2 * FX  # 212-column table window
TROWS = H * WT  # patch-table rows per core
NP = N // NCORES  # 2048 stations per core, exact
T = NP // 128  # 16 tiles
GRP = 2  # tiles per MLP group (256 stations)
NG = T // GRP
PE_TILES = (12, 13, 14, 15)  # tiles combined on PE via diag matmuls
TPE = len(PE_TILES)

_PROG_CACHE = {}


def _f32(x):
    return np.float32(x)


def _host_route(station_coords):
    """Replicate the reference index math in f32."""
    lat = np.asarray(station_coords[0, :, 0], dtype=np.float32)
    lon = np.asarray(station_coords[0, :, 1], dtype=np.float32)
    lat_n = lat / _f32(90.0)
    lon_n = lon / _f32(180.0)
    ix = np.clip((lon_n + _f32(1.0)) * _f32(0.5) * _f32(W - 1), _f32(0.0), _f32(W - 1))
    iy = np.clip((lat_n + _f32(1.0)) * _f32(0.5) * _f32(H - 1), _f32(0.0), _f32(H - 1))
    ix0f = np.floor(ix)
    iy0f = np.floor(iy)
    wx = (ix - ix0f).astype(np.float32)
    wy = (iy - iy0f).astype(np.float32)
    ix0 = ix0f.astype(np.int64)
    iy0 = iy0f.astype(np.int64)
    one = _f32(1.0)
    c00 = (one - wx) * (one - wy)
    c01 = wx * (one - wy)
    c10 = (one - wx) * wy
    c11 = wx * wy
    return ix0, iy0, (c00, c01, c10, c11)


def _host_tables(grid_features):
    """Global (H, W, C) bf16 grid + per-core patch-table windows."""
    g = np.asarray(grid_features[0], dtype=np.float32)  # (C, H, W)
    gt = np.transpose(g, (1, 2, 0)).astype(ml_dtypes.bfloat16)  # (H, W, C)
    # x+1 / y+1 with border clip
    gx1 = np.concatenate([gt[:, 1:, :], gt[:, W - 1 : W, :]], axis=1)
    gy1 = np.concatenate([gt[1:, :, :], gt[H - 1 : H, :, :]], axis=0)
    gx1y1 = np.concatenate([gy1[:, 1:, :], gy1[:, W - 1 : W, :]], axis=1)
    los = [min(max(c * COLS - FX, 0), W - WT) for c in range(NCORES)]
    tables = []
    for c in range(NCORES):
        lo = los[c]
        p = np.empty((H, WT, 4 * C), dtype=ml_dtypes.bfloat16)
        p[:, :, 0:C] = gt[:, lo : lo + WT]
        p[:, :, C : 2 * C] = gx1[:, lo : lo + WT]
        p[:, :, 2 * C : 3 * C] = gy1[:, lo : lo + WT]
        p[:, :, 3 * C : 4 * C] = gx1y1[:, lo : lo + WT]
        tables.append(p.reshape(TROWS, 4 * C))
    return tables, los


def _build_program():
    import concourse.bacc as bacc
    import concourse.bass as bass
    import concourse.mybir as mybir
    from concourse.tile import TileContext

    f32 = mybir.dt.float32
    bf16 = mybir.dt.bfloat16
    i32 = mybir.dt.int32

    nc = bacc.Bacc(
        "TRN2",
        target_bir_lowering=False,
        debug=False,
        dynamic_dma_scratch_size=49152,
    )

    tbl = nc.dram_tensor("tbl", [TROWS, 4 * C], bf16, kind="ExternalInput")
    idx = nc.dram_tensor("idx", [128, T], i32, kind="ExternalInput")
    cof = nc.dram_tensor("cof", [128, 4 * T], f32, kind="ExternalInput")
    w1 = nc.dram_tensor("w1t", [C, C], bf16, kind="ExternalInput")
    w2 = nc.dram_tensor("w2t", [C, C], bf16, kind="ExternalInput")
    bia = nc.dram_tensor("bia", [128, 4], f32, kind="ExternalInput")
    idn = nc.dram_tensor("idn", [128, 128], bf16, kind="ExternalInput")
    dgm = nc.dram_tensor("dgm", [128, TPE * 4 * 128], bf16, kind="ExternalInput")
    out = nc.dram_tensor("out", [2, 128, NP], bf16, kind="ExternalOutput")

    with TileContext(nc) as tc:
        with (
            tc.tile_pool(name="const", bufs=1) as cpool,
            tc.tile_pool(name="gat", bufs=1) as gpool,
            tc.tile_pool(name="sm", bufs=8) as spool,
            tc.tile_pool(name="xs", bufs=3) as xpool,
            tc.tile_pool(name="hs", bufs=3) as hpool,
            tc.tile_pool(name="ys", bufs=3) as ypool,
            tc.tile_pool(name="px", bufs=2, space="PSUM") as pxp,
            tc.tile_pool(name="ph", bufs=3, space="PSUM") as php,
            tc.tile_pool(name="py", bufs=3, space="PSUM") as pyp,
        ):
            idx_sb = cpool.tile([128, T], i32)
            nc.sync.dma_start(out=idx_sb[:, 0:4], in_=idx[:, 0:4])
            nc.sync.dma_start(out=idx_sb[:, 4:T], in_=idx[:, 4:T])
            cof_sb = cpool.tile([128, 4 * T], f32)
            nc.sync.dma_start(out=cof_sb[:], in_=cof[:])
            w1_sb = cpool.tile([128, 2 * C], bf16)
            nc.sync.dma_start(out=w1_sb[:, 0:C], in_=w1[0:128, :])
            nc.sync.dma_start(out=w1_sb[:, C : 2 * C], in_=w1[128:256, :])
            w2_sb = cpool.tile([128, 2 * C], bf16)
            nc.sync.dma_start(out=w2_sb[:, 0:C], in_=w2[0:128, :])
            nc.sync.dma_start(out=w2_sb[:, C : 2 * C], in_=w2[128:256, :])
            bia_sb = cpool.tile([128, 4], f32)
            nc.sync.dma_start(out=bia_sb[:], in_=bia[:])
            idn_sb = cpool.tile([128, 128], bf16)
            nc.sync.dma_start(out=idn_sb[:], in_=idn[:])
            dgm_sb = cpool.tile([128, TPE * 4 * 128], bf16)
            nc.sync.dma_start(out=dgm_sb[:], in_=dgm[:])

            # all gathers issued up-front; enough buffers that the GpSimd
            # queue never stalls (this is the serial critical path)
            gts = []
            for t in range(T):
                gt_t = gpool.tile([128, 1024], bf16, name=f"gt{t}")
                nc.gpsimd.indirect_dma_start(
                    out=gt_t[:],
                    out_offset=None,
                    in_=tbl[:],
                    in_offset=bass.IndirectOffsetOnAxis(
                        ap=idx_sb[:, t : t + 1], axis=0
                    ),
                )
                gts.append(gt_t)

            for g in range(NG):
                pe_grp = g * GRP in PE_TILES
                px = pxp.tile([128, 512], f32 if pe_grp else bf16, name="px")
                for tt in range(GRP):
                    t = g * GRP + tt
                    gt_t = gts[t]
                    if t in PE_TILES:
                        # PE path: x^T chunk = sum_q v_q^T @ diag(c_q)
                        td = PE_TILES.index(t)
                        for ch in range(2):
                            for j in range(4):
                                nc.tensor.matmul(
                                    out=px[:, ch * 256 + tt * 128 : ch * 256 + (tt + 1) * 128],
                                    lhsT=gt_t[:, j * 256 + ch * 128 : j * 256 + (ch + 1) * 128],
                                    rhs=dgm_sb[:, (td * 4 + j) * 128 : (td * 4 + j + 1) * 128],
                                    start=(j == 0),
                                    stop=(j == 3),
                                )
                    else:
                        sm = spool.tile([128, 256], bf16, name="sm")
                        for j in range(4):
                            vj = gt_t[:, j * 256 : (j + 1) * 256]
                            cj = cof_sb[:, j * T + t : j * T + t + 1]
                            if j == 0:
                                nc.vector.tensor_scalar_mul(sm[:], vj, cj)
                            else:
                                nc.vector.scalar_tensor_tensor(
                                    out=sm[:],
                                    in0=vj,
                                    scalar=cj,
                                    in1=sm[:],
                                    op0=mybir.AluOpType.mult,
                                    op1=mybir.AluOpType.add,
                                )
                        for ch in range(2):
                            nc.tensor.transpose(
                                out=px[:, ch * 256 + tt * 128 : ch * 256 + (tt + 1) * 128],
                                in_=sm[:, ch * 128 : (ch + 1) * 128],
                                identity=idn_sb[:],
                            )
                xss = xpool.tile([128, 512], bf16, name="xs")
                if g >= NG - 2:
                    nc.vector.tensor_copy(xss[:], px[:])
                else:
                    nc.scalar.activation(
                        out=xss[:],
                        in_=px[:],
                        func=mybir.ActivationFunctionType.Copy,
                    )
                ph = php.tile([128, 512], f32, name="ph")
                for m in range(2):
                    for k in range(2):
                        nc.tensor.matmul(
                            out=ph[:, m * 256 : (m + 1) * 256],
                            lhsT=w1_sb[:, k * C + m * 128 : k * C + (m + 1) * 128],
                            rhs=xss[:, k * 256 : (k + 1) * 256],
                            start=(k == 0),
                            stop=(k == 1),
                        )
                hss = hpool.tile([128, 512], bf16, name="hs")
                for m in range(2):
                    nc.scalar.activation(
                        out=hss[:, m * 256 : (m + 1) * 256],
                        in_=ph[:, m * 256 : (m + 1) * 256],
                        func=mybir.ActivationFunctionType.Gelu,
                        bias=bia_sb[:, m : m + 1],
                        scale=1.0,
                    )
                py = pyp.tile([128, 512], f32, name="py")
                for m in range(2):
                    for k in range(2):
                        nc.tensor.matmul(
                            out=py[:, m * 256 : (m + 1) * 256],
                            lhsT=w2_sb[:, k * C + m * 128 : k * C + (m + 1) * 128],
                            rhs=hss[:, k * 256 : (k + 1) * 256],
                            start=(k == 0),
                            stop=(k == 1),
                        )
                ys = ypool.tile([128, 512], bf16, name="ys")
                col = g * GRP * 128
                for m in range(2):
                    if g >= NG - 2:
                        nc.vector.tensor_scalar_add(
                            ys[:, m * 256 : (m + 1) * 256],
                            py[:, m * 256 : (m + 1) * 256],
                            bia_sb[:, 2 + m : 3 + m],
                        )
                    else:
                        nc.scalar.activation(
                            out=ys[:, m * 256 : (m + 1) * 256],
                            in_=py[:, m * 256 : (m + 1) * 256],
                            func=mybir.ActivationFunctionType.Identity,
                            bias=bia_sb[:, 2 + m : 3 + m],
                            scale=1.0,
                        )
                    nc.sync.dma_start(
                        out=out[m][:, col : col + 256], in_=ys[:, m * 256 : (m + 1) * 256]
                    )
    return nc


def _make_in_maps(grid_features, station_coords, W1, b1, W2, b2):
    ix0, iy0, cjs = _host_route(station_coords)
    tables, los = _host_tables(grid_features)

    order = np.argsort(ix0, kind="stable")
    w1t = np.ascontiguousarray(np.asarray(W1, np.float32).T).astype(ml_dtypes.bfloat16)
    w2t = np.ascontiguousarray(np.asarray(W2, np.float32).T).astype(ml_dtypes.bfloat16)
    bia = np.zeros((128, 4), np.float32)
    bia[:, 0] = b1[0:128]
    bia[:, 1] = b1[128:256]
    bia[:, 2] = b2[0:128]
    bia[:, 3] = b2[128:256]
    idn = np.eye(128, dtype=ml_dtypes.bfloat16)

    in_maps = []
    sids_per_core = []
    for c in range(NCORES):
        sids = order[c * NP : (c + 1) * NP]
        sids_per_core.append(sids)
        xl = ix0[sids] - los[c]
        assert xl.min() >= 0 and xl.max() < WT, (
            f"core {c}: station lon outside table window "
            f"({xl.min()}..{xl.max()} vs 0..{WT - 1})"
        )
        rows = (iy0[sids] * WT + xl).astype(np.int32)
        idx_arr = np.ascontiguousarray(rows.reshape(T, 128).T)  # [128, T]
        cof_arr = np.ascontiguousarray(
            np.concatenate(
                [cjs[j][sids].reshape(T, 128).T for j in range(4)], axis=1
            )
        ).astype(np.float32)
        dgm = np.zeros((128, TPE * 4 * 128), np.float32)
        ar = np.arange(128)
        for td in range(TPE):
            tpe = PE_TILES[td]
            tile_sids = sids[tpe * 128 : (tpe + 1) * 128]
            for j in range(4):
                dgm[ar, (td * 4 + j) * 128 + ar] = cjs[j][tile_sids]
        dgm = dgm.astype(ml_dtypes.bfloat16)
        in_maps.append(
            {
                "tbl": tables[c],
                "idx": idx_arr,
                "cof": cof_arr,
                "w1t": w1t,
                "w2t": w2t,
                "bia": bia,
                "idn": idn,
                "dgm": dgm,
            }
        )
    return in_maps, sids_per_core


LAST_RUN_INFO = {}


def _install_ntff_shim():
    """This container's antenv lacks axon_hooks; provide the NTFF profile
    hook via the same ctypes path trn_boot would have used."""
    import sys
    import types

    try:
        import antenv.axon_hooks  # noqa: F401

        return
    except ImportError:
        pass
    from trn_agent_boot.trn_boot import _ntff_profile_via_ctypes

    hook = _ntff_profile_via_ctypes("/opt/axon/libaxon_pjrt.so")
    mod = types.ModuleType("antenv.axon_hooks")
    mod.get_axon_ntff_profile_hook = lambda: hook
    mod.set_axon_ntff_profile_hook = lambda h: None
    sys.modules["antenv.axon_hooks"] = mod


def kernel(grid_features, station_coords, W1, b1, W2, b2):
    in_maps, sids_per_core = _make_in_maps(
        grid_features, station_coords, W1, b1, W2, b2
    )

    if "prog" not in _PROG_CACHE:
        _PROG_CACHE["prog"] = _build_program()
    nc = _PROG_CACHE["prog"]

    if os.environ.get("GRIDSTN_SIM"):
        outs = _run_sim(nc, in_maps)
    else:
        from concourse.bass_utils import run_bass_kernel_spmd

        trace = bool(os.environ.get("GRIDSTN_TRACE"))
        if trace:
            _install_ntff_shim()
        if not nc.is_finalized():
            nc.finalize()
        res = run_bass_kernel_spmd(nc, in_maps, list(range(NCORES)), trace=trace)
        LAST_RUN_INFO["exec_time_ns"] = res.exec_time_ns
        LAST_RUN_INFO["mean_exec_time_ns"] = res.mean_exec_time_ns
        LAST_RUN_INFO["profile_json"] = res.profile_json
        outs = [r["out"] for r in res.results]

    result = np.zeros((N, C), np.float32)
    for c in range(NCORES):
        y = np.asarray(outs[c]).astype(np.float32).reshape(2 * 128, NP)
        result[sids_per_core[c]] = y.T
    return result.reshape(B, N, C)


def _run_sim(nc, in_maps):
    from concourse import bass_interp

    outs = []
    for c in range(NCORES):
        sim = bass_interp.MultiCoreSim(nc, 1)
        for name, arr in in_maps[c].items():
            sim.cores[0].tensor(name)[:] = arr
        sim.simulate()
        LAST_RUN_INFO["sim_time_ns"] = sim.cores[0].time
        outs.append(np.array(sim.cores[0].tensor("out")))
        if os.environ.get("GRIDSTN_SIM_ONE_CORE"):
            outs = outs + [outs[0]] * (NCORES - 1)
            break
    return outs


# revision 20
# speedup vs baseline: 1.1573x; 1.1573x over previous
"""GridToStation Trainium2 kernel (bf16 patch-gather version).

Strategy:
  - Host: transpose grid (C,H,W) -> (H,W,C), cast bf16, and build per-core
    2x2 PATCH tables: row (y, xl) = [g[y,x], g[y,x1], g[y1,x], g[y1,x1]]
    (1024 bf16 = 2KB), where each core's table is a 212-column lon window
    around its 180-column band (x1/y1 border-clipped at build time).
  - Host: replicate the reference's f32 index math; sort stations by ix0 and
    cut every N/8 -> EXACTLY 2048 stations per core (fits each core's window
    because station lon is near-uniform; asserted). No padding anywhere.
  - Device (per core, SPMD over 8 cores), 16 tiles of 128 stations:
      * one indirect (gather) DMA per tile pulls 128 patch rows (2KB each).
        One descriptor per station is the minimum serial GpSimd/SWDGE work
        (~1.1us per call fixed) -- this is the kernel's critical path.
      * bilinear combine in [station, C] bf16: ACT seed (scale=c00) + 3 DVE
        scalar_tensor_tensor fused multiply-adds.
      * PE transpose (bf16) -> PSUM f32 -> ACT copy to bf16 x^T tiles.
      * MLP in [C, station] layout, groups of 512 stations: W1/W2 bf16
        matmuls (PSUM f32), GELU+b1 on ACT, b2 add on DVE, bf16 out.
      * per-group output DMA (overlapped), 2x[128,512] bf16 per group.
  - Host: upcast bf16 -> f32 and inverse-permute to original station order.
"""

import os

import numpy as np
import ml_dtypes

B, C, H, W, N = 1, 256, 721, 1440, 16384
NCORES = 8
COLS = W // NCORES  # 180 owned columns per core
FX = 16  # window flex columns each side
WT = COLS + 2 * FX  # 212-column table window
TROWS = H * WT  # patch-table rows per core
NP = N // NCORES  # 2048 stations per core, exact
T = NP // 128  # 16 tiles
GRP = 2  # tiles per MLP group (256 stations)
NG = T // GRP
PE_TILES = (12, 13, 14, 15)  # tiles combined on PE via diag matmuls
TPE = len(PE_TILES)

_PROG_CACHE = {}


def _f32(x):
    return np.float32(x)


def _host_route(station_coords):
    """Replicate the reference index math in f32."""
    lat = np.asarray(station_coords[0, :, 0], dtype=np.float32)
    lon = np.asarray(station_coords[0, :, 1], dtype=np.float32)
    lat_n = lat / _f32(90.0)
    lon_n = lon / _f32(180.0)
    ix = np.clip((lon_n + _f32(1.0)) * _f32(0.5) * _f32(W - 1), _f32(0.0), _f32(W - 1))
    iy = np.clip((lat_n + _f32(1.0)) * _f32(0.5) * _f32(H - 1), _f32(0.0), _f32(H - 1))
    ix0f = np.floor(ix)
    iy0f = np.floor(iy)
    wx = (ix - ix0f).astype(np.float32)
    wy = (iy - iy0f).astype(np.float32)
    ix0 = ix0f.astype(np.int64)
    iy0 = iy0f.astype(np.int64)
    one = _f32(1.0)
    c00 = (one - wx) * (one - wy)
    c01 = wx * (one - wy)
    c10 = (one - wx) * wy
    c11 = wx * wy
    return ix0, iy0, (c00, c01, c10, c11)


def _host_tables(grid_features):
    """Global (H, W, C) bf16 grid + per-core patch-table windows."""
    g = np.asarray(grid_features[0], dtype=np.float32)  # (C, H, W)
    gt = np.transpose(g, (1, 2, 0)).astype(ml_dtypes.bfloat16)  # (H, W, C)
    # x+1 / y+1 with border clip
    gx1 = np.concatenate([gt[:, 1:, :], gt[:, W - 1 : W, :]], axis=1)
    gy1 = np.concatenate([gt[1:, :, :], gt[H - 1 : H, :, :]], axis=0)
    gx1y1 = np.concatenate([gy1[:, 1:, :], gy1[:, W - 1 : W, :]], axis=1)
    los = [min(max(c * COLS - FX, 0), W - WT) for c in range(NCORES)]
    tables = []
    for c in range(NCORES):
        lo = los[c]
        p = np.empty((H, WT, 4 * C), dtype=ml_dtypes.bfloat16)
        p[:, :, 0:C] = gt[:, lo : lo + WT]
        p[:, :, C : 2 * C] = gx1[:, lo : lo + WT]
        p[:, :, 2 * C : 3 * C] = gy1[:, lo : lo + WT]
        p[:, :, 3 * C : 4 * C] = gx1y1[:, lo : lo + WT]
        tables.append(p.reshape(TROWS, 4 * C))
    return tables, los


def _build_program():
    import concourse.bacc as bacc
    import concourse.bass as bass
    import concourse.mybir as mybir
    from concourse.tile import TileContext

    f32 = mybir.dt.float32
    bf16 = mybir.dt.bfloat16
    i32 = mybir.dt.int32

    nc = bacc.Bacc(
        "TRN2",
        target_bir_lowering=False,
        debug=False,
        dynamic_dma_scratch_size=49152,
    )

    tbl = nc.dram_tensor("tbl", [TROWS, 4 * C], bf16, kind="ExternalInput")
    idx = nc.dram_tensor("idx", [128, T], i32, kind="ExternalInput")
    cof = nc.dram_tensor("cof", [128, 4 * T], f32, kind="ExternalInput")
    w1 = nc.dram_tensor("w1t", [C, C], bf16, kind="ExternalInput")
    w2 = nc.dram_tensor("w2t", [C, C], bf16, kind="ExternalInput")
    bia = nc.dram_tensor("bia", [128, 4], f32, kind="ExternalInput")
    idn = nc.dram_tensor("idn", [128, 128], bf16, kind="ExternalInput")
    dgm = nc.dram_tensor("dgm", [128, TPE * 4 * 128], bf16, kind="ExternalInput")
    out = nc.dram_tensor("out", [2, 128, NP], bf16, kind="ExternalOutput")

    with TileContext(nc) as tc:
        with (
            tc.tile_pool(name="const", bufs=1) as cpool,
            tc.tile_pool(name="gat", bufs=1) as gpool,
            tc.tile_pool(name="sm", bufs=8) as spool,
            tc.tile_pool(name="xs", bufs=3) as xpool,
            tc.tile_pool(name="hs", bufs=3) as hpool,
            tc.tile_pool(name="ys", bufs=3) as ypool,
            tc.tile_pool(name="px", bufs=2, space="PSUM") as pxp,
            tc.tile_pool(name="ph", bufs=3, space="PSUM") as php,
            tc.tile_pool(name="py", bufs=3, space="PSUM") as pyp,
        ):
            idx_sb = cpool.tile([128, T], i32)
            nc.sync.dma_start(out=idx_sb[:], in_=idx[:])
            cof_sb = cpool.tile([128, 4 * T], f32)
            nc.sync.dma_start(out=cof_sb[:], in_=cof[:])
            w1_sb = cpool.tile([128, 2 * C], bf16)
            nc.sync.dma_start(out=w1_sb[:, 0:C], in_=w1[0:128, :])
            nc.sync.dma_start(out=w1_sb[:, C : 2 * C], in_=w1[128:256, :])
            w2_sb = cpool.tile([128, 2 * C], bf16)
            nc.sync.dma_start(out=w2_sb[:, 0:C], in_=w2[0:128, :])
            nc.sync.dma_start(out=w2_sb[:, C : 2 * C], in_=w2[128:256, :])
            bia_sb = cpool.tile([128, 4], f32)
            nc.sync.dma_start(out=bia_sb[:], in_=bia[:])
            idn_sb = cpool.tile([128, 128], bf16)
            nc.sync.dma_start(out=idn_sb[:], in_=idn[:])
            dgm_sb = cpool.tile([128, TPE * 4 * 128], bf16)
            nc.sync.dma_start(out=dgm_sb[:], in_=dgm[:])

            # all gathers issued up-front; enough buffers that the GpSimd
            # queue never stalls (this is the serial critical path)
            gts = []
            for t in range(T):
                gt_t = gpool.tile([128, 1024], bf16, name=f"gt{t}")
                nc.gpsimd.indirect_dma_start(
                    out=gt_t[:],
                    out_offset=None,
                    in_=tbl[:],
                    in_offset=bass.IndirectOffsetOnAxis(
                        ap=idx_sb[:, t : t + 1], axis=0
                    ),
                )
                gts.append(gt_t)

            for g in range(NG):
                pe_grp = g * GRP in PE_TILES
                px = pxp.tile([128, 512], f32 if pe_grp else bf16, name="px")
                for tt in range(GRP):
                    t = g * GRP + tt
                    gt_t = gts[t]
                    if t in PE_TILES:
                        # PE path: x^T chunk = sum_q v_q^T @ diag(c_q)
                        td = PE_TILES.index(t)
                        for ch in range(2):
                            for j in range(4):
                                nc.tensor.matmul(
                                    out=px[:, ch * 256 + tt * 128 : ch * 256 + (tt + 1) * 128],
                                    lhsT=gt_t[:, j * 256 + ch * 128 : j * 256 + (ch + 1) * 128],
                                    rhs=dgm_sb[:, (td * 4 + j) * 128 : (td * 4 + j + 1) * 128],
                                    start=(j == 0),
                                    stop=(j == 3),
                                )
                    else:
                        sm = spool.tile([128, 256], bf16, name="sm")
                        for j in range(4):
                            vj = gt_t[:, j * 256 : (j + 1) * 256]
                            cj = cof_sb[:, j * T + t : j * T + t + 1]
                            if j == 0:
                                nc.vector.tensor_scalar_mul(sm[:], vj, cj)
                            else:
                                nc.vector.scalar_tensor_tensor(
                                    out=sm[:],
                                    in0=vj,
                                    scalar=cj,
                                    in1=sm[:],
                                    op0=mybir.AluOpType.mult,
                                    op1=mybir.AluOpType.add,
                                )
                        for ch in range(2):
                            nc.tensor.transpose(
                                out=px[:, ch * 256 + tt * 128 : ch * 256 + (tt + 1) * 128],
                                in_=sm[:, ch * 128 : (ch + 1) * 128],
                                identity=idn_sb[:],
                            )
                xss = xpool.tile([128, 512], bf16, name="xs")
                if g >= NG - 2:
                    nc.vector.tensor_copy(xss[:], px[:])
                else:
                    nc.scalar.activation(
                        out=xss[:],
                        in_=px[:],
                        func=mybir.ActivationFunctionType.Copy,
                    )
                ph = php.tile([128, 512], f32, name="ph")
                for m in range(2):
                    for k in range(2):
                        nc.tensor.matmul(
                            out=ph[:, m * 256 : (m + 1) * 256],
                            lhsT=w1_sb[:, k * C + m * 128 : k * C + (m + 1) * 128],
                            rhs=xss[:, k * 256 : (k + 1) * 256],
                            start=(k == 0),
                            stop=(k == 1),
                        )
                hss = hpool.tile([128, 512], bf16, name="hs")
                for m in range(2):
                    nc.scalar.activation(
                        out=hss[:, m * 256 : (m + 1) * 256],
                        in_=ph[:, m * 256 : (m + 1) * 256],
                        func=mybir.ActivationFunctionType.Gelu,
                        bias=bia_sb[:, m : m + 1],
                        scale=1.0,
                    )
                py = pyp.tile([128, 512], f32, name="py")
                for m in range(2):
                    for k in range(2):
                        nc.tensor.matmul(
                            out=py[:, m * 256 : (m + 1) * 256],
                            lhsT=w2_sb[:, k * C + m * 128 : k * C + (m + 1) * 128],
                            rhs=hss[:, k * 256 : (k + 1) * 256],
                            start=(k == 0),
                            stop=(k == 1),
                        )
                ys = ypool.tile([128, 512], bf16, name="ys")
                col = g * GRP * 128
                for m in range(2):
                    if g >= NG - 2:
                        nc.vector.tensor_scalar_add(
                            ys[:, m * 256 : (m + 1) * 256],
                            py[:, m * 256 : (m + 1) * 256],
                            bia_sb[:, 2 + m : 3 + m],
                        )
                    else:
                        nc.scalar.activation(
                            out=ys[:, m * 256 : (m + 1) * 256],
                            in_=py[:, m * 256 : (m + 1) * 256],
                            func=mybir.ActivationFunctionType.Identity,
                            bias=bia_sb[:, 2 + m : 3 + m],
                            scale=1.0,
                        )
                    nc.sync.dma_start(
                        out=out[m][:, col : col + 256], in_=ys[:, m * 256 : (m + 1) * 256]
                    )
    return nc


def _make_in_maps(grid_features, station_coords, W1, b1, W2, b2):
    ix0, iy0, cjs = _host_route(station_coords)
    tables, los = _host_tables(grid_features)

    order = np.argsort(ix0, kind="stable")
    w1t = np.ascontiguousarray(np.asarray(W1, np.float32).T).astype(ml_dtypes.bfloat16)
    w2t = np.ascontiguousarray(np.asarray(W2, np.float32).T).astype(ml_dtypes.bfloat16)
    bia = np.zeros((128, 4), np.float32)
    bia[:, 0] = b1[0:128]
    bia[:, 1] = b1[128:256]
    bia[:, 2] = b2[0:128]
    bia[:, 3] = b2[128:256]
    idn = np.eye(128, dtype=ml_dtypes.bfloat16)

    in_maps = []
    sids_per_core = []
    for c in range(NCORES):
        sids = order[c * NP : (c + 1) * NP]
        sids_per_core.append(sids)
        xl = ix0[sids] - los[c]
        assert xl.min() >= 0 and xl.max() < WT, (
            f"core {c}: station lon outside table window "
            f"({xl.min()}..{xl.max()} vs 0..{WT - 1})"
        )
        rows = (iy0[sids] * WT + xl).astype(np.int32)
        idx_arr = np.ascontiguousarray(rows.reshape(T, 128).T)  # [128, T]
        cof_arr = np.ascontiguousarray(
            np.concatenate(
                [cjs[j][sids].reshape(T, 128).T for j in range(4)], axis=1
            )
        ).astype(np.float32)
        dgm = np.zeros((128, TPE * 4 * 128), np.float32)
        ar = np.arange(128)
        for td in range(TPE):
            tpe = PE_TILES[td]
            tile_sids = sids[tpe * 128 : (tpe + 1) * 128]
            for j in range(4):
                dgm[ar, (td * 4 + j) * 128 + ar] = cjs[j][tile_sids]
        dgm = dgm.astype(ml_dtypes.bfloat16)
        in_maps.append(
            {
                "tbl": tables[c],
                "idx": idx_arr,
                "cof": cof_arr,
                "w1t": w1t,
                "w2t": w2t,
                "bia": bia,
                "idn": idn,
                "dgm": dgm,
            }
        )
    return in_maps, sids_per_core


LAST_RUN_INFO = {}


def _install_ntff_shim():
    """This container's antenv lacks axon_hooks; provide the NTFF profile
    hook via the same ctypes path trn_boot would have used."""
    import sys
    import types

    try:
        import antenv.axon_hooks  # noqa: F401

        return
    except ImportError:
        pass
    from trn_agent_boot.trn_boot import _ntff_profile_via_ctypes

    hook = _ntff_profile_via_ctypes("/opt/axon/libaxon_pjrt.so")
    mod = types.ModuleType("antenv.axon_hooks")
    mod.get_axon_ntff_profile_hook = lambda: hook
    mod.set_axon_ntff_profile_hook = lambda h: None
    sys.modules["antenv.axon_hooks"] = mod


def kernel(grid_features, station_coords, W1, b1, W2, b2):
    in_maps, sids_per_core = _make_in_maps(
        grid_features, station_coords, W1, b1, W2, b2
    )

    if "prog" not in _PROG_CACHE:
        _PROG_CACHE["prog"] = _build_program()
    nc = _PROG_CACHE["prog"]

    if os.environ.get("GRIDSTN_SIM"):
        outs = _run_sim(nc, in_maps)
    else:
        from concourse.bass_utils import run_bass_kernel_spmd

        trace = bool(os.environ.get("GRIDSTN_TRACE"))
        if trace:
            _install_ntff_shim()
        if not nc.is_finalized():
            nc.finalize()
        res = run_bass_kernel_spmd(nc, in_maps, list(range(NCORES)), trace=trace)
        LAST_RUN_INFO["exec_time_ns"] = res.exec_time_ns
        LAST_RUN_INFO["mean_exec_time_ns"] = res.mean_exec_time_ns
        LAST_RUN_INFO["profile_json"] = res.profile_json
        outs = [r["out"] for r in res.results]

    result = np.zeros((N, C), np.float32)
    for c in range(NCORES):
        y = np.asarray(outs[c]).astype(np.float32).reshape(2 * 128, NP)
        result[sids_per_core[c]] = y.T
    return result.reshape(B, N, C)


def _run_sim(nc, in_maps):
    from concourse import bass_interp

    outs = []
    for c in range(NCORES):
        sim = bass_interp.MultiCoreSim(nc, 1)
        for name, arr in in_maps[c].items():
            sim.cores[0].tensor(name)[:] = arr
        sim.simulate()
        LAST_RUN_INFO["sim_time_ns"] = sim.cores[0].time
        outs.append(np.array(sim.cores[0].tensor("out")))
        if os.environ.get("GRIDSTN_SIM_ONE_CORE"):
            outs = outs + [outs[0]] * (NCORES - 1)
            break
    return outs


# revision 21
# speedup vs baseline: 1.1613x; 1.0035x over previous
"""GridToStation Trainium2 kernel (bf16 patch-gather version).

Strategy:
  - Host: transpose grid (C,H,W) -> (H,W,C), cast bf16, and build per-core
    2x2 PATCH tables: row (y, xl) = [g[y,x], g[y,x1], g[y1,x], g[y1,x1]]
    (1024 bf16 = 2KB), where each core's table is a 212-column lon window
    around its 180-column band (x1/y1 border-clipped at build time).
  - Host: replicate the reference's f32 index math; sort stations by ix0 and
    cut every N/8 -> EXACTLY 2048 stations per core (fits each core's window
    because station lon is near-uniform; asserted). No padding anywhere.
  - Device (per core, SPMD over 8 cores), 16 tiles of 128 stations:
      * one indirect (gather) DMA per tile pulls 128 patch rows (2KB each).
        One descriptor per station is the minimum serial GpSimd/SWDGE work
        (~1.1us per call fixed) -- this is the kernel's critical path.
      * bilinear combine in [station, C] bf16: ACT seed (scale=c00) + 3 DVE
        scalar_tensor_tensor fused multiply-adds.
      * PE transpose (bf16) -> PSUM f32 -> ACT copy to bf16 x^T tiles.
      * MLP in [C, station] layout, groups of 512 stations: W1/W2 bf16
        matmuls (PSUM f32), GELU+b1 on ACT, b2 add on DVE, bf16 out.
      * per-group output DMA (overlapped), 2x[128,512] bf16 per group.
  - Host: upcast bf16 -> f32 and inverse-permute to original station order.
"""

import os

import numpy as np
import ml_dtypes

B, C, H, W, N = 1, 256, 721, 1440, 16384
NCORES = 8
COLS = W // NCORES  # 180 owned columns per core
FX = 16  # window flex columns each side
WT = COLS + 2 * FX  # 212-column table window
TROWS = H * WT  # patch-table rows per core
NP = N // NCORES  # 2048 stations per core, exact
T = NP // 128  # 16 tiles
GRP = 2  # tiles per MLP group (256 stations)
NG = T // GRP
PE_TILES = (12, 13, 14, 15)  # tiles combined on PE via diag matmuls
TPE = len(PE_TILES)

_PROG_CACHE = {}


def _f32(x):
    return np.float32(x)


def _host_route(station_coords):
    """Replicate the reference index math in f32."""
    lat = np.asarray(station_coords[0, :, 0], dtype=np.float32)
    lon = np.asarray(station_coords[0, :, 1], dtype=np.float32)
    lat_n = lat / _f32(90.0)
    lon_n = lon / _f32(180.0)
    ix = np.clip((lon_n + _f32(1.0)) * _f32(0.5) * _f32(W - 1), _f32(0.0), _f32(W - 1))
    iy = np.clip((lat_n + _f32(1.0)) * _f32(0.5) * _f32(H - 1), _f32(0.0), _f32(H - 1))
    ix0f = np.floor(ix)
    iy0f = np.floor(iy)
    wx = (ix - ix0f).astype(np.float32)
    wy = (iy - iy0f).astype(np.float32)
    ix0 = ix0f.astype(np.int64)
    iy0 = iy0f.astype(np.int64)
    one = _f32(1.0)
    c00 = (one - wx) * (one - wy)
    c01 = wx * (one - wy)
    c10 = (one - wx) * wy
    c11 = wx * wy
    return ix0, iy0, (c00, c01, c10, c11)


def _host_tables(grid_features):
    """Global (H, W, C) bf16 grid + per-core patch-table windows."""
    g = np.asarray(grid_features[0], dtype=np.float32)  # (C, H, W)
    gt = np.transpose(g, (1, 2, 0)).astype(ml_dtypes.bfloat16)  # (H, W, C)
    # x+1 / y+1 with border clip
    gx1 = np.concatenate([gt[:, 1:, :], gt[:, W - 1 : W, :]], axis=1)
    gy1 = np.concatenate([gt[1:, :, :], gt[H - 1 : H, :, :]], axis=0)
    gx1y1 = np.concatenate([gy1[:, 1:, :], gy1[:, W - 1 : W, :]], axis=1)
    los = [min(max(c * COLS - FX, 0), W - WT) for c in range(NCORES)]
    tables = []
    for c in range(NCORES):
        lo = los[c]
        p = np.empty((H, WT, 4 * C), dtype=ml_dtypes.bfloat16)
        p[:, :, 0:C] = gt[:, lo : lo + WT]
        p[:, :, C : 2 * C] = gx1[:, lo : lo + WT]
        p[:, :, 2 * C : 3 * C] = gy1[:, lo : lo + WT]
        p[:, :, 3 * C : 4 * C] = gx1y1[:, lo : lo + WT]
        tables.append(p.reshape(TROWS, 4 * C))
    return tables, los


def _build_program():
    import concourse.bacc as bacc
    import concourse.bass as bass
    import concourse.mybir as mybir
    from concourse.tile import TileContext

    f32 = mybir.dt.float32
    bf16 = mybir.dt.bfloat16
    i32 = mybir.dt.int32

    nc = bacc.Bacc(
        "TRN2",
        target_bir_lowering=False,
        debug=False,
        dynamic_dma_scratch_size=49152,
        num_swdge_queues=2,
    )

    tbl = nc.dram_tensor("tbl", [TROWS, 4 * C], bf16, kind="ExternalInput")
    idx = nc.dram_tensor("idx", [128, T], i32, kind="ExternalInput")
    cof = nc.dram_tensor("cof", [128, 4 * T], f32, kind="ExternalInput")
    w1 = nc.dram_tensor("w1t", [C, C], bf16, kind="ExternalInput")
    w2 = nc.dram_tensor("w2t", [C, C], bf16, kind="ExternalInput")
    bia = nc.dram_tensor("bia", [128, 4], f32, kind="ExternalInput")
    idn = nc.dram_tensor("idn", [128, 128], bf16, kind="ExternalInput")
    dgm = nc.dram_tensor("dgm", [128, TPE * 4 * 128], bf16, kind="ExternalInput")
    out = nc.dram_tensor("out", [2, 128, NP], bf16, kind="ExternalOutput")

    with TileContext(nc) as tc:
        with (
            tc.tile_pool(name="const", bufs=1) as cpool,
            tc.tile_pool(name="gat", bufs=1) as gpool,
            tc.tile_pool(name="sm", bufs=8) as spool,
            tc.tile_pool(name="xs", bufs=3) as xpool,
            tc.tile_pool(name="hs", bufs=3) as hpool,
            tc.tile_pool(name="ys", bufs=3) as ypool,
            tc.tile_pool(name="px", bufs=2, space="PSUM") as pxp,
            tc.tile_pool(name="ph", bufs=3, space="PSUM") as php,
            tc.tile_pool(name="py", bufs=3, space="PSUM") as pyp,
        ):
            idx_sb = cpool.tile([128, T], i32)
            nc.sync.dma_start(out=idx_sb[:], in_=idx[:])
            cof_sb = cpool.tile([128, 4 * T], f32)
            nc.sync.dma_start(out=cof_sb[:], in_=cof[:])
            w1_sb = cpool.tile([128, 2 * C], bf16)
            nc.sync.dma_start(out=w1_sb[:, 0:C], in_=w1[0:128, :])
            nc.sync.dma_start(out=w1_sb[:, C : 2 * C], in_=w1[128:256, :])
            w2_sb = cpool.tile([128, 2 * C], bf16)
            nc.sync.dma_start(out=w2_sb[:, 0:C], in_=w2[0:128, :])
            nc.sync.dma_start(out=w2_sb[:, C : 2 * C], in_=w2[128:256, :])
            bia_sb = cpool.tile([128, 4], f32)
            nc.sync.dma_start(out=bia_sb[:], in_=bia[:])
            idn_sb = cpool.tile([128, 128], bf16)
            nc.sync.dma_start(out=idn_sb[:], in_=idn[:])
            dgm_sb = cpool.tile([128, TPE * 4 * 128], bf16)
            nc.sync.dma_start(out=dgm_sb[:], in_=dgm[:])

            # all gathers issued up-front; enough buffers that the GpSimd
            # queue never stalls (this is the serial critical path)
            gts = []
            for t in range(T):
                gt_t = gpool.tile([128, 1024], bf16, name=f"gt{t}")
                bi = nc.gpsimd.indirect_dma_start(
                    out=gt_t[:],
                    out_offset=None,
                    in_=tbl[:],
                    in_offset=bass.IndirectOffsetOnAxis(
                        ap=idx_sb[:, t : t + 1], axis=0
                    ),
                )
                if t % 2:
                    bi.ins.queue = "qPoolDynamic1"
                gts.append(gt_t)

            for g in range(NG):
                pe_grp = g * GRP in PE_TILES
                px = pxp.tile([128, 512], f32 if pe_grp else bf16, name="px")
                for tt in range(GRP):
                    t = g * GRP + tt
                    gt_t = gts[t]
                    if t in PE_TILES:
                        # PE path: x^T chunk = sum_q v_q^T @ diag(c_q)
                        td = PE_TILES.index(t)
                        for ch in range(2):
                            for j in range(4):
                                nc.tensor.matmul(
                                    out=px[:, ch * 256 + tt * 128 : ch * 256 + (tt + 1) * 128],
                                    lhsT=gt_t[:, j * 256 + ch * 128 : j * 256 + (ch + 1) * 128],
                                    rhs=dgm_sb[:, (td * 4 + j) * 128 : (td * 4 + j + 1) * 128],
                                    start=(j == 0),
                                    stop=(j == 3),
                                )
                    else:
                        sm = spool.tile([128, 256], bf16, name="sm")
                        for j in range(4):
                            vj = gt_t[:, j * 256 : (j + 1) * 256]
                            cj = cof_sb[:, j * T + t : j * T + t + 1]
                            if j == 0:
                                nc.vector.tensor_scalar_mul(sm[:], vj, cj)
                            else:
                                nc.vector.scalar_tensor_tensor(
                                    out=sm[:],
                                    in0=vj,
                                    scalar=cj,
                                    in1=sm[:],
                                    op0=mybir.AluOpType.mult,
                                    op1=mybir.AluOpType.add,
                                )
                        for ch in range(2):
                            nc.tensor.transpose(
                                out=px[:, ch * 256 + tt * 128 : ch * 256 + (tt + 1) * 128],
                                in_=sm[:, ch * 128 : (ch + 1) * 128],
                                identity=idn_sb[:],
                            )
                xss = xpool.tile([128, 512], bf16, name="xs")
                if g >= NG - 2:
                    nc.vector.tensor_copy(xss[:], px[:])
                else:
                    nc.scalar.activation(
                        out=xss[:],
                        in_=px[:],
                        func=mybir.ActivationFunctionType.Copy,
                    )
                ph = php.tile([128, 512], f32, name="ph")
                for m in range(2):
                    for k in range(2):
                        nc.tensor.matmul(
                            out=ph[:, m * 256 : (m + 1) * 256],
                            lhsT=w1_sb[:, k * C + m * 128 : k * C + (m + 1) * 128],
                            rhs=xss[:, k * 256 : (k + 1) * 256],
                            start=(k == 0),
                            stop=(k == 1),
                        )
                hss = hpool.tile([128, 512], bf16, name="hs")
                for m in range(2):
                    nc.scalar.activation(
                        out=hss[:, m * 256 : (m + 1) * 256],
                        in_=ph[:, m * 256 : (m + 1) * 256],
                        func=mybir.ActivationFunctionType.Gelu,
                        bias=bia_sb[:, m : m + 1],
                        scale=1.0,
                    )
                py = pyp.tile([128, 512], f32, name="py")
                for m in range(2):
                    for k in range(2):
                        nc.tensor.matmul(
                            out=py[:, m * 256 : (m + 1) * 256],
                            lhsT=w2_sb[:, k * C + m * 128 : k * C + (m + 1) * 128],
                            rhs=hss[:, k * 256 : (k + 1) * 256],
                            start=(k == 0),
                            stop=(k == 1),
                        )
                ys = ypool.tile([128, 512], bf16, name="ys")
                col = g * GRP * 128
                for m in range(2):
                    if g >= NG - 2:
                        nc.vector.tensor_scalar_add(
                            ys[:, m * 256 : (m + 1) * 256],
                            py[:, m * 256 : (m + 1) * 256],
                            bia_sb[:, 2 + m : 3 + m],
                        )
                    else:
                        nc.scalar.activation(
                            out=ys[:, m * 256 : (m + 1) * 256],
                            in_=py[:, m * 256 : (m + 1) * 256],
                            func=mybir.ActivationFunctionType.Identity,
                            bias=bia_sb[:, 2 + m : 3 + m],
                            scale=1.0,
                        )
                    nc.sync.dma_start(
                        out=out[m][:, col : col + 256], in_=ys[:, m * 256 : (m + 1) * 256]
                    )
    return nc


def _make_in_maps(grid_features, station_coords, W1, b1, W2, b2):
    ix0, iy0, cjs = _host_route(station_coords)
    tables, los = _host_tables(grid_features)

    order = np.argsort(ix0, kind="stable")
    w1t = np.ascontiguousarray(np.asarray(W1, np.float32).T).astype(ml_dtypes.bfloat16)
    w2t = np.ascontiguousarray(np.asarray(W2, np.float32).T).astype(ml_dtypes.bfloat16)
    bia = np.zeros((128, 4), np.float32)
    bia[:, 0] = b1[0:128]
    bia[:, 1] = b1[128:256]
    bia[:, 2] = b2[0:128]
    bia[:, 3] = b2[128:256]
    idn = np.eye(128, dtype=ml_dtypes.bfloat16)

    in_maps = []
    sids_per_core = []
    for c in range(NCORES):
        sids = order[c * NP : (c + 1) * NP]
        sids_per_core.append(sids)
        xl = ix0[sids] - los[c]
        assert xl.min() >= 0 and xl.max() < WT, (
            f"core {c}: station lon outside table window "
            f"({xl.min()}..{xl.max()} vs 0..{WT - 1})"
        )
        rows = (iy0[sids] * WT + xl).astype(np.int32)
        idx_arr = np.ascontiguousarray(rows.reshape(T, 128).T)  # [128, T]
        cof_arr = np.ascontiguousarray(
            np.concatenate(
                [cjs[j][sids].reshape(T, 128).T for j in range(4)], axis=1
            )
        ).astype(np.float32)
        dgm = np.zeros((128, TPE * 4 * 128), np.float32)
        ar = np.arange(128)
        for td in range(TPE):
            tpe = PE_TILES[td]
            tile_sids = sids[tpe * 128 : (tpe + 1) * 128]
            for j in range(4):
                dgm[ar, (td * 4 + j) * 128 + ar] = cjs[j][tile_sids]
        dgm = dgm.astype(ml_dtypes.bfloat16)
        in_maps.append(
            {
                "tbl": tables[c],
                "idx": idx_arr,
                "cof": cof_arr,
                "w1t": w1t,
                "w2t": w2t,
                "bia": bia,
                "idn": idn,
                "dgm": dgm,
            }
        )
    return in_maps, sids_per_core


LAST_RUN_INFO = {}


def _install_ntff_shim():
    """This container's antenv lacks axon_hooks; provide the NTFF profile
    hook via the same ctypes path trn_boot would have used."""
    import sys
    import types

    try:
        import antenv.axon_hooks  # noqa: F401

        return
    except ImportError:
        pass
    from trn_agent_boot.trn_boot import _ntff_profile_via_ctypes

    hook = _ntff_profile_via_ctypes("/opt/axon/libaxon_pjrt.so")
    mod = types.ModuleType("antenv.axon_hooks")
    mod.get_axon_ntff_profile_hook = lambda: hook
    mod.set_axon_ntff_profile_hook = lambda h: None
    sys.modules["antenv.axon_hooks"] = mod


def kernel(grid_features, station_coords, W1, b1, W2, b2):
    in_maps, sids_per_core = _make_in_maps(
        grid_features, station_coords, W1, b1, W2, b2
    )

    if "prog" not in _PROG_CACHE:
        _PROG_CACHE["prog"] = _build_program()
    nc = _PROG_CACHE["prog"]

    if os.environ.get("GRIDSTN_SIM"):
        outs = _run_sim(nc, in_maps)
    else:
        from concourse.bass_utils import run_bass_kernel_spmd

        trace = bool(os.environ.get("GRIDSTN_TRACE"))
        if trace:
            _install_ntff_shim()
        if not nc.is_finalized():
            nc.finalize()
        res = run_bass_kernel_spmd(nc, in_maps, list(range(NCORES)), trace=trace)
        LAST_RUN_INFO["exec_time_ns"] = res.exec_time_ns
        LAST_RUN_INFO["mean_exec_time_ns"] = res.mean_exec_time_ns
        LAST_RUN_INFO["profile_json"] = res.profile_json
        outs = [r["out"] for r in res.results]

    result = np.zeros((N, C), np.float32)
    for c in range(NCORES):
        y = np.asarray(outs[c]).astype(np.float32).reshape(2 * 128, NP)
        result[sids_per_core[c]] = y.T
    return result.reshape(B, N, C)


def _run_sim(nc, in_maps):
    from concourse import bass_interp

    outs = []
    for c in range(NCORES):
        sim = bass_interp.MultiCoreSim(nc, 1)
        for name, arr in in_maps[c].items():
            sim.cores[0].tensor(name)[:] = arr
        sim.simulate()
        LAST_RUN_INFO["sim_time_ns"] = sim.cores[0].time
        outs.append(np.array(sim.cores[0].tensor("out")))
        if os.environ.get("GRIDSTN_SIM_ONE_CORE"):
            outs = outs + [outs[0]] * (NCORES - 1)
            break
    return outs


# revision 22
# speedup vs baseline: 1.1724x; 1.0095x over previous
"""GridToStation Trainium2 kernel (bf16 patch-gather version).

Strategy:
  - Host: transpose grid (C,H,W) -> (H,W,C), cast bf16, and build per-core
    2x2 PATCH tables: row (y, xl) = [g[y,x], g[y,x1], g[y1,x], g[y1,x1]]
    (1024 bf16 = 2KB), where each core's table is a 212-column lon window
    around its 180-column band (x1/y1 border-clipped at build time).
  - Host: replicate the reference's f32 index math; sort stations by ix0 and
    cut every N/8 -> EXACTLY 2048 stations per core (fits each core's window
    because station lon is near-uniform; asserted). No padding anywhere.
  - Device (per core, SPMD over 8 cores), 16 tiles of 128 stations:
      * one indirect (gather) DMA per tile pulls 128 patch rows (2KB each).
        One descriptor per station is the minimum serial GpSimd/SWDGE work
        (~1.1us per call fixed) -- this is the kernel's critical path.
      * bilinear combine in [station, C] bf16: ACT seed (scale=c00) + 3 DVE
        scalar_tensor_tensor fused multiply-adds.
      * PE transpose (bf16) -> PSUM f32 -> ACT copy to bf16 x^T tiles.
      * MLP in [C, station] layout, groups of 512 stations: W1/W2 bf16
        matmuls (PSUM f32), GELU+b1 on ACT, b2 add on DVE, bf16 out.
      * per-group output DMA (overlapped), 2x[128,512] bf16 per group.
  - Host: upcast bf16 -> f32 and inverse-permute to original station order.
"""

import os

import numpy as np
import ml_dtypes

B, C, H, W, N = 1, 256, 721, 1440, 16384
NCORES = 8
COLS = W // NCORES  # 180 owned columns per core
FX = 16  # window flex columns each side
WT = COLS + 2 * FX  # 212-column table window
TROWS = H * WT  # patch-table rows per core
NP = N // NCORES  # 2048 stations per core, exact
T = NP // 128  # 16 tiles
GRP = 2  # tiles per MLP group (256 stations)
NG = T // GRP
PE_TILES = (12, 13, 14, 15)  # tiles combined on PE via diag matmuls
TPE = len(PE_TILES)

_PROG_CACHE = {}


def _f32(x):
    return np.float32(x)


def _host_route(station_coords):
    """Replicate the reference index math in f32."""
    lat = np.asarray(station_coords[0, :, 0], dtype=np.float32)
    lon = np.asarray(station_coords[0, :, 1], dtype=np.float32)
    lat_n = lat / _f32(90.0)
    lon_n = lon / _f32(180.0)
    ix = np.clip((lon_n + _f32(1.0)) * _f32(0.5) * _f32(W - 1), _f32(0.0), _f32(W - 1))
    iy = np.clip((lat_n + _f32(1.0)) * _f32(0.5) * _f32(H - 1), _f32(0.0), _f32(H - 1))
    ix0f = np.floor(ix)
    iy0f = np.floor(iy)
    wx = (ix - ix0f).astype(np.float32)
    wy = (iy - iy0f).astype(np.float32)
    ix0 = ix0f.astype(np.int64)
    iy0 = iy0f.astype(np.int64)
    one = _f32(1.0)
    c00 = (one - wx) * (one - wy)
    c01 = wx * (one - wy)
    c10 = (one - wx) * wy
    c11 = wx * wy
    return ix0, iy0, (c00, c01, c10, c11)


def _host_tables(grid_features):
    """Global (H, W, C) bf16 grid + per-core patch-table windows."""
    g = np.asarray(grid_features[0], dtype=np.float32)  # (C, H, W)
    gt = np.transpose(g, (1, 2, 0)).astype(ml_dtypes.bfloat16)  # (H, W, C)
    # x+1 / y+1 with border clip
    gx1 = np.concatenate([gt[:, 1:, :], gt[:, W - 1 : W, :]], axis=1)
    gy1 = np.concatenate([gt[1:, :, :], gt[H - 1 : H, :, :]], axis=0)
    gx1y1 = np.concatenate([gy1[:, 1:, :], gy1[:, W - 1 : W, :]], axis=1)
    los = [min(max(c * COLS - FX, 0), W - WT) for c in range(NCORES)]
    tables = []
    for c in range(NCORES):
        lo = los[c]
        p = np.empty((H, WT, 4 * C), dtype=ml_dtypes.bfloat16)
        p[:, :, 0:C] = gt[:, lo : lo + WT]
        p[:, :, C : 2 * C] = gx1[:, lo : lo + WT]
        p[:, :, 2 * C : 3 * C] = gy1[:, lo : lo + WT]
        p[:, :, 3 * C : 4 * C] = gx1y1[:, lo : lo + WT]
        tables.append(p.reshape(TROWS, 4 * C))
    return tables, los


def _build_program():
    import concourse.bacc as bacc
    import concourse.bass as bass
    import concourse.mybir as mybir
    from concourse.tile import TileContext

    f32 = mybir.dt.float32
    bf16 = mybir.dt.bfloat16
    i32 = mybir.dt.int32

    nc = bacc.Bacc(
        "TRN2",
        target_bir_lowering=False,
        debug=False,
        dynamic_dma_scratch_size=49152,
        num_swdge_queues=2,
    )

    tbl = nc.dram_tensor("tbl", [TROWS, 4 * C], bf16, kind="ExternalInput")
    idx = nc.dram_tensor("idx", [128, T], i32, kind="ExternalInput")
    cof = nc.dram_tensor("cof", [128, 4 * T], f32, kind="ExternalInput")
    w1 = nc.dram_tensor("w1t", [C, C], bf16, kind="ExternalInput")
    w2 = nc.dram_tensor("w2t", [C, C], bf16, kind="ExternalInput")
    bia = nc.dram_tensor("bia", [128, 4], f32, kind="ExternalInput")
    idn = nc.dram_tensor("idn", [128, 128], bf16, kind="ExternalInput")
    dgm = nc.dram_tensor("dgm", [128, TPE * 4 * 128], bf16, kind="ExternalInput")
    out = nc.dram_tensor("out", [2, 128, NP], bf16, kind="ExternalOutput")

    with TileContext(nc) as tc:
        with (
            tc.tile_pool(name="const", bufs=1) as cpool,
            tc.tile_pool(name="gat", bufs=1) as gpool,
            tc.tile_pool(name="sm", bufs=10) as spool,
            tc.tile_pool(name="xs", bufs=3) as xpool,
            tc.tile_pool(name="hs", bufs=3) as hpool,
            tc.tile_pool(name="ys", bufs=4) as ypool,
            tc.tile_pool(name="px", bufs=2, space="PSUM") as pxp,
            tc.tile_pool(name="ph", bufs=3, space="PSUM") as php,
            tc.tile_pool(name="py", bufs=3, space="PSUM") as pyp,
        ):
            idx_sb = cpool.tile([128, T], i32)
            nc.gpsimd.dma_start(out=idx_sb[:], in_=idx[:])
            cof_sb = cpool.tile([128, 4 * T], f32)
            nc.sync.dma_start(out=cof_sb[:], in_=cof[:])
            w1_sb = cpool.tile([128, 2 * C], bf16)
            nc.sync.dma_start(out=w1_sb[:, 0:C], in_=w1[0:128, :])
            nc.sync.dma_start(out=w1_sb[:, C : 2 * C], in_=w1[128:256, :])
            w2_sb = cpool.tile([128, 2 * C], bf16)
            nc.sync.dma_start(out=w2_sb[:, 0:C], in_=w2[0:128, :])
            nc.sync.dma_start(out=w2_sb[:, C : 2 * C], in_=w2[128:256, :])
            bia_sb = cpool.tile([128, 4], f32)
            nc.sync.dma_start(out=bia_sb[:], in_=bia[:])
            idn_sb = cpool.tile([128, 128], bf16)
            nc.sync.dma_start(out=idn_sb[:], in_=idn[:])
            dgm_sb = cpool.tile([128, TPE * 4 * 128], bf16)
            nc.sync.dma_start(out=dgm_sb[:], in_=dgm[:])

            # all gathers issued up-front; enough buffers that the GpSimd
            # queue never stalls (this is the serial critical path)
            gts = []
            for t in range(T):
                gt_t = gpool.tile([128, 1024], bf16, name=f"gt{t}")
                bi = nc.gpsimd.indirect_dma_start(
                    out=gt_t[:],
                    out_offset=None,
                    in_=tbl[:],
                    in_offset=bass.IndirectOffsetOnAxis(
                        ap=idx_sb[:, t : t + 1], axis=0
                    ),
                )
                if t % 2:
                    bi.ins.queue = "qPoolDynamic1"
                gts.append(gt_t)

            for g in range(NG):
                pe_grp = g * GRP in PE_TILES
                px = pxp.tile([128, 512], f32 if pe_grp else bf16, name="px")
                for tt in range(GRP):
                    t = g * GRP + tt
                    gt_t = gts[t]
                    if t in PE_TILES:
                        # PE path: x^T chunk = sum_q v_q^T @ diag(c_q)
                        td = PE_TILES.index(t)
                        for ch in range(2):
                            for j in range(4):
                                nc.tensor.matmul(
                                    out=px[:, ch * 256 + tt * 128 : ch * 256 + (tt + 1) * 128],
                                    lhsT=gt_t[:, j * 256 + ch * 128 : j * 256 + (ch + 1) * 128],
                                    rhs=dgm_sb[:, (td * 4 + j) * 128 : (td * 4 + j + 1) * 128],
                                    start=(j == 0),
                                    stop=(j == 3),
                                )
                    else:
                        sm = spool.tile([128, 256], bf16, name="sm")
                        for j in range(4):
                            vj = gt_t[:, j * 256 : (j + 1) * 256]
                            cj = cof_sb[:, j * T + t : j * T + t + 1]
                            if j == 0:
                                nc.vector.tensor_scalar_mul(sm[:], vj, cj)
                            else:
                                nc.vector.scalar_tensor_tensor(
                                    out=sm[:],
                                    in0=vj,
                                    scalar=cj,
                                    in1=sm[:],
                                    op0=mybir.AluOpType.mult,
                                    op1=mybir.AluOpType.add,
                                )
                        for ch in range(2):
                            nc.tensor.transpose(
                                out=px[:, ch * 256 + tt * 128 : ch * 256 + (tt + 1) * 128],
                                in_=sm[:, ch * 128 : (ch + 1) * 128],
                                identity=idn_sb[:],
                            )
                xss = xpool.tile([128, 512], bf16, name="xs")
                if g >= NG - 2:
                    nc.vector.tensor_copy(xss[:], px[:])
                else:
                    nc.scalar.activation(
                        out=xss[:],
                        in_=px[:],
                        func=mybir.ActivationFunctionType.Copy,
                    )
                ph = php.tile([128, 512], f32, name="ph")
                for m in range(2):
                    for k in range(2):
                        nc.tensor.matmul(
                            out=ph[:, m * 256 : (m + 1) * 256],
                            lhsT=w1_sb[:, k * C + m * 128 : k * C + (m + 1) * 128],
                            rhs=xss[:, k * 256 : (k + 1) * 256],
                            start=(k == 0),
                            stop=(k == 1),
                        )
                hss = hpool.tile([128, 512], bf16, name="hs")
                for m in range(2):
                    nc.scalar.activation(
                        out=hss[:, m * 256 : (m + 1) * 256],
                        in_=ph[:, m * 256 : (m + 1) * 256],
                        func=mybir.ActivationFunctionType.Gelu,
                        bias=bia_sb[:, m : m + 1],
                        scale=1.0,
                    )
                py = pyp.tile([128, 512], f32, name="py")
                for m in range(2):
                    for k in range(2):
                        nc.tensor.matmul(
                            out=py[:, m * 256 : (m + 1) * 256],
                            lhsT=w2_sb[:, k * C + m * 128 : k * C + (m + 1) * 128],
                            rhs=hss[:, k * 256 : (k + 1) * 256],
                            start=(k == 0),
                            stop=(k == 1),
                        )
                ys = ypool.tile([128, 512], bf16, name="ys")
                col = g * GRP * 128
                for m in range(2):
                    if g >= NG - 2:
                        nc.vector.tensor_scalar_add(
                            ys[:, m * 256 : (m + 1) * 256],
                            py[:, m * 256 : (m + 1) * 256],
                            bia_sb[:, 2 + m : 3 + m],
                        )
                    else:
                        nc.scalar.activation(
                            out=ys[:, m * 256 : (m + 1) * 256],
                            in_=py[:, m * 256 : (m + 1) * 256],
                            func=mybir.ActivationFunctionType.Identity,
                            bias=bia_sb[:, 2 + m : 3 + m],
                            scale=1.0,
                        )
                    nc.sync.dma_start(
                        out=out[m][:, col : col + 256], in_=ys[:, m * 256 : (m + 1) * 256]
                    )
    return nc


def _make_in_maps(grid_features, station_coords, W1, b1, W2, b2):
    ix0, iy0, cjs = _host_route(station_coords)
    tables, los = _host_tables(grid_features)

    order = np.argsort(ix0, kind="stable")
    w1t = np.ascontiguousarray(np.asarray(W1, np.float32).T).astype(ml_dtypes.bfloat16)
    w2t = np.ascontiguousarray(np.asarray(W2, np.float32).T).astype(ml_dtypes.bfloat16)
    bia = np.zeros((128, 4), np.float32)
    bia[:, 0] = b1[0:128]
    bia[:, 1] = b1[128:256]
    bia[:, 2] = b2[0:128]
    bia[:, 3] = b2[128:256]
    idn = np.eye(128, dtype=ml_dtypes.bfloat16)

    in_maps = []
    sids_per_core = []
    for c in range(NCORES):
        sids = order[c * NP : (c + 1) * NP]
        sids_per_core.append(sids)
        xl = ix0[sids] - los[c]
        assert xl.min() >= 0 and xl.max() < WT, (
            f"core {c}: station lon outside table window "
            f"({xl.min()}..{xl.max()} vs 0..{WT - 1})"
        )
        rows = (iy0[sids] * WT + xl).astype(np.int32)
        idx_arr = np.ascontiguousarray(rows.reshape(T, 128).T)  # [128, T]
        cof_arr = np.ascontiguousarray(
            np.concatenate(
                [cjs[j][sids].reshape(T, 128).T for j in range(4)], axis=1
            )
        ).astype(np.float32)
        dgm = np.zeros((128, TPE * 4 * 128), np.float32)
        ar = np.arange(128)
        for td in range(TPE):
            tpe = PE_TILES[td]
            tile_sids = sids[tpe * 128 : (tpe + 1) * 128]
            for j in range(4):
                dgm[ar, (td * 4 + j) * 128 + ar] = cjs[j][tile_sids]
        dgm = dgm.astype(ml_dtypes.bfloat16)
        in_maps.append(
            {
                "tbl": tables[c],
                "idx": idx_arr,
                "cof": cof_arr,
                "w1t": w1t,
                "w2t": w2t,
                "bia": bia,
                "idn": idn,
                "dgm": dgm,
            }
        )
    return in_maps, sids_per_core


LAST_RUN_INFO = {}


def _install_ntff_shim():
    """This container's antenv lacks axon_hooks; provide the NTFF profile
    hook via the same ctypes path trn_boot would have used."""
    import sys
    import types

    try:
        import antenv.axon_hooks  # noqa: F401

        return
    except ImportError:
        pass
    from trn_agent_boot.trn_boot import _ntff_profile_via_ctypes

    hook = _ntff_profile_via_ctypes("/opt/axon/libaxon_pjrt.so")
    mod = types.ModuleType("antenv.axon_hooks")
    mod.get_axon_ntff_profile_hook = lambda: hook
    mod.set_axon_ntff_profile_hook = lambda h: None
    sys.modules["antenv.axon_hooks"] = mod


def kernel(grid_features, station_coords, W1, b1, W2, b2):
    in_maps, sids_per_core = _make_in_maps(
        grid_features, station_coords, W1, b1, W2, b2
    )

    if "prog" not in _PROG_CACHE:
        _PROG_CACHE["prog"] = _build_program()
    nc = _PROG_CACHE["prog"]

    if os.environ.get("GRIDSTN_SIM"):
        outs = _run_sim(nc, in_maps)
    else:
        from concourse.bass_utils import run_bass_kernel_spmd

        trace = bool(os.environ.get("GRIDSTN_TRACE"))
        if trace:
            _install_ntff_shim()
        if not nc.is_finalized():
            nc.finalize()
        res = run_bass_kernel_spmd(nc, in_maps, list(range(NCORES)), trace=trace)
        LAST_RUN_INFO["exec_time_ns"] = res.exec_time_ns
        LAST_RUN_INFO["mean_exec_time_ns"] = res.mean_exec_time_ns
        LAST_RUN_INFO["profile_json"] = res.profile_json
        outs = [r["out"] for r in res.results]

    result = np.zeros((N, C), np.float32)
    for c in range(NCORES):
        y = np.asarray(outs[c]).astype(np.float32).reshape(2 * 128, NP)
        result[sids_per_core[c]] = y.T
    return result.reshape(B, N, C)


def _run_sim(nc, in_maps):
    from concourse import bass_interp

    outs = []
    for c in range(NCORES):
        sim = bass_interp.MultiCoreSim(nc, 1)
        for name, arr in in_maps[c].items():
            sim.cores[0].tensor(name)[:] = arr
        sim.simulate()
        LAST_RUN_INFO["sim_time_ns"] = sim.cores[0].time
        outs.append(np.array(sim.cores[0].tensor("out")))
        if os.environ.get("GRIDSTN_SIM_ONE_CORE"):
            outs = outs + [outs[0]] * (NCORES - 1)
            break
    return outs
